# revision 3
# baseline (speedup 1.0000x reference)
"""ChaosSSMCore (diag mode) Trainium2 kernel.

Problem: B=4, S=4096, D=1024, fp32.
    delta  = softplus(x @ Wd.T); decay = exp(-delta * sigmoid(log_a))
    update = sigmoid(x @ Ws.T) * tanh(x @ Wi.T)
    gate   = sigmoid(x @ Wg.T)
    state_t = decay_t * state_{t-1} + update_t        (scan over S, elementwise in D)
    y      = (gate * states) @ Wo.T

Sharding: 8 cores = 4 batches x 2 halves of the D output dim. Each core
computes its 512-channel slice of the 4 input projections in [e, s] layout
(channels on partitions, sequence on the free axis), applies activations on
the scalar engine, runs the hardware tensor_tensor_scan (state = d*s + u along
the free dim) on the vector engine, gates, and computes a partial output GEMM
against its 512 columns of W_out. Host sums the two partials per batch and
transposes back.

All matmul operands are bf16 (host-side conversion; PSUM accumulation stays
fp32): the PE streams bf16 and fp32r at the same 1 row/cycle, but bf16 weight
loads are shorter (less unhidden LDWEIGHTS time) and every input DMA halves.
Measured end-to-end rel err ~4e-3 vs the 2e-2 gate. The scan runs in fp32;
only the gated scan output (the moving operand of the output GEMM) and the
y partials written back to HBM are bf16.

DMA layout: every DRAM tensor is pre-arranged on the host so each SBUF
partition's data is one contiguous 8KB+ run (1KB-row descriptors measured
~23GB/s/engine from packet overhead; contiguous runs lift that). Startup
spreads the first loads across the Sync/GpSimd/Scalar DMA queues so w0, x0
and w1 stream concurrently, and a short burst of dummy matmuls on zeroed
scratch warms the PE DVFS ramp while they land. y is written back with one
DMA per sequence chunk from a batched [P, MT*NC] tile.

Activation tables: sigmoid is computed as (1 + tanh(z/2))/2 with the 1/2
input scale folded into W_select/W_gate and the output scales into W_out
(W_out/4), so every psum-draining activation (delta-Exp + 12 Tanh) lives in
the `exp_and_others` table set. softplus(z) = ln(exp(z)+1) needs Ln, which
lives elsewhere: the per-chunk [4x Ln][4x decay-Exp] block runs on SBUF
tiles only, so its two table loads sit off the psum-drain path and the PE
never waits on a table switch. Per chunk the ACT order is pinned: [4x
Exp(delta psum)] [4x Ln] [4x decay-Exp] [12x Tanh(psum)].

The last chunk's scan/gate/output GEMM run in two 256-wide halves with the
psum->bf16 casts split across the Vector and Scalar engines, so the final
HBM flush shrinks and starts earlier.
"""

import sys

if "/opt/trn_rl_repo" not in sys.path:
    sys.path.insert(0, "/opt/trn_rl_repo")

import numpy as np

# Problem constants (hardcoded per harness contract).
B, S, D = 4, 4096, 1024
P = 128           # SBUF partitions
E = D // 2        # per-core channel slice
NC = 512          # sequence chunk (= one PSUM bank of fp32)
KT = D // P       # k-tiles per input projection contraction (8)
ET = E // P       # e-tiles per core (4)
KO = E // P       # k'-tiles for the output GEMM contraction (4)
MT = D // P       # output-row tiles (8)
SC = S // NC      # sequence chunks (8)
HC = NC // 2      # half-chunk for the tail drain (256)
N_WARM = 12       # dummy warmup matmuls (PE DVFS ramp during startup DMAs)
N_CORES = 8

_CACHE = {}


def _build_program():
    import concourse.bacc as bacc
    import concourse.mybir as mybir
    import concourse.tile as tile
    from concourse.tile import add_dep_helper
    from contextlib import ExitStack

    f32 = mybir.dt.float32
    bf16 = mybir.dt.bfloat16
    AF = mybir.ActivationFunctionType
    OP = mybir.AluOpType

    nc = bacc.Bacc("TRN2", target_bir_lowering=False)

    # Host pre-arranges everything partition-major so each partition's slice
    # of any single DMA is contiguous DRAM.
    xt = nc.declare_dram_parameter("xt", [P, SC, KT, NC], bf16, isOutput=False)
    w4 = nc.declare_dram_parameter("w4", [4, P, KT * E], bf16, isOutput=False)
    wo = nc.declare_dram_parameter("wo", [P, KO * D], bf16, isOutput=False)
    na = nc.declare_dram_parameter("na", [P, ET], f32, isOutput=False)
    yt = nc.declare_dram_parameter("yt", [P, SC, MT, NC], bf16, isOutput=True)

    with tile.TileContext(nc) as tc, ExitStack() as ctx:
        wpool = ctx.enter_context(tc.tile_pool(name="w", bufs=1))
        xpool = ctx.enter_context(tc.tile_pool(name="x", bufs=2))
        ppd = ctx.enter_context(tc.tile_pool(name="ppd", bufs=3, space="PSUM"))
        pp = ctx.enter_context(tc.tile_pool(name="pp", bufs=3, space="PSUM"))
        pyp = ctx.enter_context(tc.tile_pool(name="pyp", bufs=2, space="PSUM"))
        dpool = ctx.enter_context(tc.tile_pool(name="dp", bufs=2))
        decpool = ctx.enter_context(tc.tile_pool(name="dec", bufs=8))
        spool = ctx.enter_context(tc.tile_pool(name="sp", bufs=4))
        tpool = ctx.enter_context(tc.tile_pool(name="tp", bufs=4))
        upool = ctx.enter_context(tc.tile_pool(name="up", bufs=3))
        stpool = ctx.enter_context(tc.tile_pool(name="stp", bufs=6))
        gpool = ctx.enter_context(tc.tile_pool(name="gp", bufs=4))
        gdpool = ctx.enter_context(tc.tile_pool(name="gdp", bufs=8))
        ypool = ctx.enter_context(tc.tile_pool(name="yp", bufs=2))

        # Pin the ACT instruction order to the emission order so the
        # scheduler cannot move a psum-draining activation behind a
        # table-switching phase from another chunk.
        last_act = [None]

        def act(*args, **kwargs):
            h = nc.scalar.activation(*args, **kwargs)
            if last_act[0] is not None:
                add_dep_helper(h.ins, last_act[0].ins, sync=False,
                               reason="pin ACT table phase order")
            last_act[0] = h
            return h

        # PE warmup: zeroed scratch + dummy matmuls queued ahead of the
        # first real group so the DVFS ramp runs while the startup DMAs
        # land (first real matmuls then issue at full clock).
        xz = wpool.tile([P, NC], bf16, name="xz", tag="xz")
        wz = wpool.tile([P, P], bf16, name="wz", tag="wz")
        nc.vector.memset(xz[:, :], 0.0)
        nc.vector.memset(wz[:, :], 0.0)

        # Startup loads, spread across three DMA queues so w0 / x0 / w1
        # stream concurrently:
        #   sync:   w0, w2, x(1)
        #   gpsimd: x0, w3
        #   scalar: w1, na, wo
        w_sb = [None] * 4
        for q in range(4):
            w_sb[q] = wpool.tile([P, KT * E], bf16, name=f"w{q}_sb", tag=f"w{q}")
        x_next = xpool.tile([P, KT * NC], bf16, name="x_sb", tag="x")
        na_sb = wpool.tile([P, ET], f32, name="na_sb", tag="na")
        wo_sb = wpool.tile([P, KO * D], bf16, name="wo_sb", tag="wo")
        nc.sync.dma_start(w_sb[0][:, :], w4[0])
        nc.gpsimd.dma_start(
            x_next.rearrange("p (k s) -> p k s", k=KT), xt[:, 0])
        nc.scalar.dma_start(w_sb[1][:, :], w4[1])
        nc.sync.dma_start(w_sb[2][:, :], w4[2])
        nc.gpsimd.dma_start(w_sb[3][:, :], w4[3])
        nc.scalar.dma_start(na_sb[:, :], na[:, :])
        nc.scalar.dma_start(wo_sb[:, :], wo[:, :])

        def load_x(c):
            x_sb = xpool.tile([P, KT * NC], bf16, name="x_sb", tag="x")
            eng = nc.sync if c == 1 else nc.gpsimd
            eng.dma_start(x_sb.rearrange("p (k s) -> p k s", k=KT), xt[:, c])
            return x_sb

        x1 = load_x(1)

        # Dummy warmup matmuls (zeros in, never read back).
        warm_ps = ppd.tile([P, NC], f32, name="warm", tag="ppd")
        for _ in range(N_WARM):
            nc.tensor.matmul(warm_ps[:, :], wz[:, :], xz[:, :],
                             start=True, stop=True)

        prev_states = [None] * ET
        gated_tiles = [[None] * ET for _ in range(SC)]
        front_state = {}

        def mm_group(ps, q, j, x_sb):
            for k in range(KT):
                nc.tensor.matmul(
                    ps[:, :],
                    w_sb[q][:, k * E + j * P: k * E + (j + 1) * P],
                    x_sb[:, k * NC:(k + 1) * NC],
                    start=(k == 0),
                    stop=(k == KT - 1),
                )

        def emit_front(c, x_sb):
            # Phase A: delta projection; Exp drains each psum as it fills.
            e1_t = []
            for j in range(ET):
                ps = ppd.tile([P, NC], f32, name="psd", tag="ppd")
                mm_group(ps, 0, j, x_sb)
                e1 = dpool.tile([P, NC], f32, name="e1", tag="e1", bufs=5)
                act(e1[:, :], ps[:, :], AF.Exp)
                e1_t.append(e1)
            # Phase L (SBUF only, absorbs both table loads): softplus tail
            # ln(e1+1), then decay = exp(-a * softplus).
            l1_t = []
            for j in range(ET):
                l1 = dpool.tile([P, NC], f32, name="l1", tag="l1", bufs=5)
                act(l1[:, :], e1_t[j][:, :], AF.Ln, bias=1.0)
                l1_t.append(l1)
            dec_t = []
            for j in range(ET):
                d = decpool.tile([P, NC], f32, name="dec", tag="dec")
                act(d[:, :], l1_t[j][:, :], AF.Exp, scale=na_sb[:, j:j + 1])
                dec_t.append(d)
            # Phase B: the three gate projections, q-outer so chunk 0 only
            # needs each weight tensor when its 32-matmul block starts
            # (matches the startup DMA arrival order). Tanh drains share
            # the Exp table set, so no load between phases.
            tS, tI, tG = [], [], []
            for q, lst, pool, nm in (
                (1, tS, spool, "tS"),
                (2, tI, tpool, "tI"),
                (3, tG, gpool, "tG"),
            ):
                for j in range(ET):
                    ps = pp.tile([P, NC], f32, name="ps", tag="pp")
                    mm_group(ps, q, j, x_sb)
                    t = pool.tile([P, NC], f32, name=nm, tag=nm)
                    act(t[:, :], ps[:, :], AF.Tanh)
                    lst.append(t)
            front_state[c] = (dec_t, tS, tI, tG)

        def phase_c(c):
            # update' = (1+tS)*tI ; scan ; gated' = (1+tG)*st  (bf16 out).
            dec_t, tS, tI, tG = front_state.pop(c)
            for j in range(ET):
                u = upool.tile([P, NC], f32, name="upd", tag="upd")
                nc.vector.scalar_tensor_tensor(
                    u[:, :], tS[j][:, :], 1.0, tI[j][:, :],
                    op0=OP.add, op1=OP.mult,
                )
                st = stpool.tile([P, NC], f32, name="st", tag="st")
                init = 0.0 if c == 0 else prev_states[j][:, NC - 1:NC]
                nc.vector.tensor_tensor_scan(
                    st[:, :], dec_t[j][:, :], u[:, :], init,
                    op0=OP.mult, op1=OP.add,
                )
                prev_states[j] = st
                g = gdpool.tile([P, NC], bf16, name="gated", tag="gated")
                nc.vector.scalar_tensor_tensor(
                    g[:, :], tG[j][:, :], 1.0, st[:, :],
                    op0=OP.add, op1=OP.mult,
                )
                gated_tiles[c][j] = g

        def emit_back(c):
            # Output GEMM for chunk c; casts collect into one [P, MT*NC]
            # tile and ship with a single contiguous DMA.
            y_big = ypool.tile([P, MT * NC], bf16, name="y_big", tag="ybig")
            for m in range(MT):
                py = pyp.tile([P, NC], f32, name="py", tag="py")
                for j in range(KO):
                    nc.tensor.matmul(
                        py[:, :],
                        wo_sb[:, j * D + m * P: j * D + (m + 1) * P],
                        gated_tiles[c][j][:, :],
                        start=(j == 0),
                        stop=(j == KO - 1),
                    )
                nc.vector.tensor_copy(
                    y_big[:, m * NC:(m + 1) * NC], py[:, :])
            nc.sync.dma_start(
                yt[:, c], y_big.rearrange("p (m s) -> p m s", m=MT))
            gated_tiles[c] = [None] * KO

        def emit_back_final(c):
            # Tail: run the last chunk's scan/gate/output GEMM in two
            # 256-wide halves so the first half's stores stream while the
            # second half is still scanning. All 8 PSUM banks are free
            # here; accumulate j-major into 8 live half-bank psums so the
            # PE streams 8 matmuls the moment each gated slice lands. The
            # psum->bf16 casts alternate between Vector and Scalar.
            dec_t, tS, tI, tG = front_state.pop(c)
            pools = [
                (pyp, "py"), (pyp, "py"), (pp, "pp"), (pp, "pp"),
                (pp, "pp"), (ppd, "ppd"), (ppd, "ppd"), (ppd, "ppd"),
            ]
            st_prev = prev_states
            for half in range(2):
                hsl = slice(half * HC, (half + 1) * HC)
                g_h = []
                for j in range(ET):
                    u = upool.tile([P, HC], f32, name="updh", tag="upd")
                    nc.vector.scalar_tensor_tensor(
                        u[:, :], tS[j][:, hsl], 1.0, tI[j][:, hsl],
                        op0=OP.add, op1=OP.mult,
                    )
                    st = stpool.tile([P, HC], f32, name="sth", tag="st")
                    init = st_prev[j][:, -1:]
                    nc.vector.tensor_tensor_scan(
                        st[:, :], dec_t[j][:, hsl], u[:, :], init,
                        op0=OP.mult, op1=OP.add,
                    )
                    st_prev[j] = st
                    g = gdpool.tile([P, HC], bf16, name="gatedh", tag="gated")
                    nc.vector.scalar_tensor_tensor(
                        g[:, :], tG[j][:, hsl], 1.0, st[:, :],
                        op0=OP.add, op1=OP.mult,
                    )
                    g_h.append(g)
                y_ps = [
                    pool.tile([P, HC], f32, name=f"pyf{m}", tag=tag)
                    for m, (pool, tag) in enumerate(pools)
                ]
                y_big = ypool.tile([P, MT * HC], bf16, name="y_bigh",
                                   tag="ybig")
                for j in range(KO - 1):
                    for m in range(MT):
                        nc.tensor.matmul(
                            y_ps[m][:, :],
                            wo_sb[:, j * D + m * P: j * D + (m + 1) * P],
                            g_h[j][:, :],
                            start=(j == 0),
                            stop=False,
                        )
                j = KO - 1
                for m in range(MT):
                    nc.tensor.matmul(
                        y_ps[m][:, :],
                        wo_sb[:, j * D + m * P: j * D + (m + 1) * P],
                        g_h[j][:, :],
                        start=False,
                        stop=True,
                    )
                    dst = y_big[:, m * HC:(m + 1) * HC]
                    if m % 2 == 0:
                        nc.vector.tensor_copy(dst, y_ps[m][:, :])
                    else:
                        nc.scalar.activation(dst, y_ps[m][:, :], AF.Copy)
                nc.sync.dma_start(
                    yt[:, c, :, hsl],
                    y_big.rearrange("p (m s) -> p m s", m=MT))

        for c in range(SC):
            x_cur = x_next
            if c == 0:
                x_next = x1
            elif c + 1 < SC:
                x_next = load_x(c + 1)
            emit_front(c, x_cur)
            if c < SC - 1:
                phase_c(c)
            if c > 0:
                emit_back(c - 1)
        emit_back_final(SC - 1)

    nc.compile()
    return nc


def _get_program():
    if "nc" not in _CACHE:
        _CACHE["nc"] = _build_program()
    return _CACHE["nc"]


def _make_in_maps(x, W_in, W_select, W_gate, W_out, W_delta, log_a):
    import ml_dtypes

    bf = ml_dtypes.bfloat16
    a = (1.0 / (1.0 + np.exp(-log_a.astype(np.float32)))).astype(np.float32)
    in_maps = []
    for c in range(N_CORES):
        b, h = divmod(c, 2)
        sl = slice(h * E, (h + 1) * E)
        # xt[p, c, k, n] = x[b, c*NC+n, k*P+p]
        xT = np.ascontiguousarray(
            x[b].T.astype(bf).reshape(KT, P, SC, NC).transpose(1, 2, 0, 3)
        )                                                       # [P,SC,KT,NC]
        # w4[q, p, k*E+e] = Wq_eff.T[k*P+p, e]
        w4 = np.ascontiguousarray(
            np.stack(
                [
                    W_delta[sl, :].T,
                    0.5 * W_select[sl, :].T,   # sigmoid via tanh(z/2)
                    W_in[sl, :].T,
                    0.5 * W_gate[sl, :].T,     # sigmoid via tanh(z/2)
                ]
            ).astype(bf).reshape(4, KT, P, E).transpose(0, 2, 1, 3)
            .reshape(4, P, KT * E)
        )
        # wo[p, j*D+d] = Wo_eff.T[j*P+p, d]
        wo = np.ascontiguousarray(
            (0.25 * W_out[:, sl].T).astype(bf)
            .reshape(KO, P, D).transpose(1, 0, 2).reshape(P, KO * D)
        )
        na_m = np.ascontiguousarray((-a[sl]).reshape(ET, P).T)  # [P, ET]
        in_maps.append({"xt": xT, "w4": w4, "wo": wo, "na": na_m})
    return in_maps


def _gather(results):
    y = np.empty((B, S, D), np.float32)
    for b in range(B):
        yT = np.zeros((D, S), np.float32)
        for r in (results[2 * b], results[2 * b + 1]):
            # yt[p, c, m, n] = y_T[m*P+p, c*NC+n]
            arr = r["yt"].astype(np.float32)
            yT += arr.transpose(2, 0, 1, 3).reshape(D, S)
        y[b] = yT.T
    return y


def kernel(x, W_in, W_select, W_gate, W_out, W_delta, log_a):
    from concourse.bass_utils import run_bass_kernel_spmd

    nc = _get_program()
    in_maps = _make_in_maps(
        np.asarray(x, np.float32),
        np.asarray(W_in, np.float32),
        np.asarray(W_select, np.float32),
        np.asarray(W_gate, np.float32),
        np.asarray(W_out, np.float32),
        np.asarray(W_delta, np.float32),
        np.asarray(log_a, np.float32),
    )
    res = run_bass_kernel_spmd(nc, in_maps, core_ids=list(range(N_CORES)))
    return _gather(res.results)


if __name__ == "__main__":
    nc = _get_program()
    print("program built OK")
